# revision 10
# baseline (speedup 1.0000x reference)
"""AdaptiveSampler Trainium2 kernel (8 NeuronCores, pure data parallel).

Computes, per batch row b:
    Q  = target_embed @ Wq.T + bq                      [B, d]
    K  = candidate_embeds @ Wk.T(+bk)  (never materialized)
    scores[b, n] = (Q[b] . K[b, n]) / sqrt(d)
    probs = 0.9 * softmax(scores) + 0.1 / N_CAND
    keys  = log(probs) + gumbel(u)
    out   = top-32 indices of keys (descending)

Key algebraic rewrite: scores[b,n] = cand[b,n,:] . Qk[b,:] + Q[b].bk, with
Qk = Q @ Wk.  The additive Q[b].bk term is constant per row, so it cancels
in softmax and is dropped.  This avoids materializing K entirely; the main
loop touches each candidate embedding element exactly once (memory bound).

Sharding: batch dim 4096 split across 8 cores (512 rows each); weights
replicated; no cross-core communication.
"""

import sys

for _p in ("/opt/trn_rl_repo",):
    if _p not in sys.path:
        sys.path.append(_p)

from contextlib import ExitStack

import numpy as np

import concourse.bass as bass
import concourse.bacc as bacc
import concourse.mybir as mybir
import concourse.tile as tile
from concourse import masks
from concourse.bass_utils import run_bass_kernel_spmd

F32 = mybir.dt.float32
U32 = mybir.dt.uint32
AF = mybir.ActivationFunctionType
OP = mybir.AluOpType
AX = mybir.AxisListType

B_FULL = 4096
N_CORES = 8
B_SHARD = B_FULL // N_CORES  # 512
D = 128
N_CAND = 512
K_OUT = 32
GAMMA = 0.1
MIX = GAMMA / N_CAND
INVSCALE = float(D) ** -0.5
NEG_BIG = -1e30


def build_nc(b_shard=B_SHARD, nch=128, gp_chunks=0, cand_bufs=2):
    """Build the single-core Bass program (SPMD across 8 cores).

    b_shard: rows handled by this core (multiple of 128).
    nch: candidate chunk size for the score accumulation loop.
    gp_chunks: how many of the n-chunks per block run on GPSIMD (rest on DVE).
    """
    assert b_shard % 128 == 0
    assert N_CAND % nch == 0
    nblk = b_shard // 128
    nchunks = N_CAND // nch

    nc = bacc.Bacc("TRN2", target_bir_lowering=False, debug=False)

    t_target = nc.dram_tensor("target_embed", [b_shard, D], F32, kind="ExternalInput")
    t_cand = nc.dram_tensor(
        "candidate_embeds", [b_shard, N_CAND, D], F32, kind="ExternalInput"
    )
    t_wq = nc.dram_tensor("Wq", [D, D], F32, kind="ExternalInput")
    t_bq = nc.dram_tensor("bq", [D], F32, kind="ExternalInput")
    t_wk = nc.dram_tensor("Wk", [D, D], F32, kind="ExternalInput")
    t_bk = nc.dram_tensor("bk", [D], F32, kind="ExternalInput")
    t_u = nc.dram_tensor("u", [b_shard, N_CAND], F32, kind="ExternalInput")
    t_out = nc.dram_tensor("out", [b_shard, K_OUT], U32, kind="ExternalOutput")

    cand_ap = t_cand.ap()
    u_ap = t_u.ap()
    out_ap = t_out.ap()

    with tile.TileContext(nc) as tc, ExitStack() as ctx:
        const_pool = ctx.enter_context(tc.tile_pool(name="const", bufs=1))
        pre_pool = ctx.enter_context(tc.tile_pool(name="pre", bufs=2))
        psum_pool = ctx.enter_context(tc.tile_pool(name="psum", bufs=2, space="PSUM"))
        cand_pool = ctx.enter_context(tc.tile_pool(name="cand", bufs=cand_bufs))
        work_pool = ctx.enter_context(tc.tile_pool(name="work", bufs=2))

        # ---------------- preamble: Qk = (target @ Wq.T + bq) @ Wk -------------
        # NOTE: every PE (TensorEngine) operand below is produced by a DVE
        # instruction.  Walrus rejects Matmult instructions that carry more
        # than one sync wait; funnelling all PE inputs through DVE collapses
        # each matmul's dependencies onto the single DVE semaphore.
        ident0 = const_pool.tile([128, 128], F32)
        masks.make_identity(nc, ident0[:])
        ident = const_pool.tile([128, 128], F32)
        nc.vector.tensor_copy(ident[:], ident0[:])

        eps_c = const_pool.tile([128, 1], F32)
        nc.gpsimd.memset(eps_c[:], 1e-20)

        wq_t = pre_pool.tile([128, D], F32, tag="wload")
        nc.scalar.dma_start(wq_t[:], t_wq.ap())
        wq_sb = const_pool.tile([128, D], F32)
        nc.vector.tensor_copy(wq_sb[:], wq_t[:])
        wk_t = pre_pool.tile([128, D], F32, tag="wload")
        nc.scalar.dma_start(wk_t[:], t_wk.ap())
        wk_sb = const_pool.tile([128, D], F32)
        nc.vector.tensor_copy(wk_sb[:], wk_t[:])
        bq_c = const_pool.tile([128, 1], F32)
        nc.scalar.dma_start(bq_c[:], t_bq.ap()[:, None])

        # transpose Wq -> wqT (lhsT for the Q projection)
        tp_ps = psum_pool.tile([128, 128], F32, tag="tp")
        nc.tensor.transpose(tp_ps[:], wq_sb[:], ident[:])
        wqT = const_pool.tile([128, D], F32)
        nc.vector.tensor_copy(wqT[:], tp_ps[:])

        # transpose target (per 128-row block) -> targetT [d, b_shard]
        targetT = const_pool.tile([128, b_shard], F32)
        for blk in range(nblk):
            tgt_t = pre_pool.tile([128, D], F32, tag="tgt")
            nc.scalar.dma_start(tgt_t[:], t_target.ap()[blk * 128 : (blk + 1) * 128, :])
            tgt_sb = pre_pool.tile([128, D], F32, tag="tgt_sb")
            nc.vector.tensor_copy(tgt_sb[:], tgt_t[:])
            tp_ps = psum_pool.tile([128, 128], F32, tag="tp")
            nc.tensor.transpose(tp_ps[:], tgt_sb[:], ident[:])
            nc.vector.tensor_copy(targetT[:, blk * 128 : (blk + 1) * 128], tp_ps[:])

        # QT[e, r] = sum_d Wq[e, d] * targetT[d, r]  (lhsT = Wq.T)
        qt_ps = psum_pool.tile([128, b_shard], F32, tag="qt")
        nc.tensor.matmul(qt_ps[:], wqT[:], targetT[:], start=True, stop=True)
        qt_sb = const_pool.tile([128, b_shard], F32)
        nc.vector.tensor_scalar_add(qt_sb[:], qt_ps[:], bq_c[:])  # + bq[e]

        # QkT[dd, r] = sum_e Wk[e, dd] * QT[e, r]  (lhsT = Wk, natural layout)
        qkt_ps = psum_pool.tile([128, b_shard], F32, tag="qt")
        nc.tensor.matmul(qkt_ps[:], wk_sb[:], qt_sb[:], start=True, stop=True)
        qkt_sb = const_pool.tile([128, b_shard], F32)
        nc.vector.tensor_copy(qkt_sb[:], qkt_ps[:])

        # Qk with rows in partitions: qk_all[p, blk*128 + d] = Qk[blk*128+p, d]
        qk_all = const_pool.tile([128, b_shard], F32)
        for blk in range(nblk):
            tp_ps = psum_pool.tile([128, 128], F32, tag="tp")
            nc.tensor.transpose(
                tp_ps[:], qkt_sb[:, blk * 128 : (blk + 1) * 128], ident[:]
            )
            nc.vector.tensor_copy(qk_all[:, blk * 128 : (blk + 1) * 128], tp_ps[:])

        # ---------------- main loop over 128-row blocks ------------------------
        for bb in range(nblk):
            r0 = bb * 128
            u_t = work_pool.tile([128, N_CAND], F32, tag="u_t")
            nc.scalar.dma_start(u_t[:], u_ap[r0 : r0 + 128, :])

            s_t = work_pool.tile([128, N_CAND], F32, tag="s_t")

            for ch in range(nchunks):
                n0 = ch * nch
                cand_t = cand_pool.tile([128, nch, D], F32, tag="cand_t")
                nc.sync.dma_start(
                    cand_t[:], cand_ap[r0 : r0 + 128, n0 : n0 + nch, :]
                )
                seg = s_t[:, n0 : n0 + nch]
                eng = nc.gpsimd if ch >= nchunks - gp_chunks else nc.vector
                # seg = sum_d cand[:, :, d] * Qk[:, d]
                eng.tensor_scalar(
                    seg, cand_t[:, :, 0], qk_all[:, r0 : r0 + 1], None, op0=OP.mult
                )
                for d in range(1, D):
                    eng.scalar_tensor_tensor(
                        seg,
                        cand_t[:, :, d],
                        qk_all[:, r0 + d : r0 + d + 1],
                        seg,
                        op0=OP.mult,
                        op1=OP.add,
                    )

            # ---- softmax -> mixed probs -> log, on the raw (unscaled) scores --
            m_t = work_pool.tile([128, 1], F32, tag="m_t")
            nc.vector.tensor_reduce(m_t[:], s_t[:], axis=AX.X, op=OP.max)
            mb_t = work_pool.tile([128, 1], F32, tag="mb_t")
            nc.vector.tensor_scalar_mul(mb_t[:], m_t[:], -INVSCALE)

            e_t = work_pool.tile([128, N_CAND], F32, tag="e_t")
            sum_t = work_pool.tile([128, 1], F32, tag="sum_t")
            # e = exp(s*invscale - max*invscale), sum_t = row sum of e
            nc.scalar.activation(
                e_t[:], s_t[:], AF.Exp, bias=mb_t[:], scale=INVSCALE,
                accum_out=sum_t[:],
            )
            r_t = work_pool.tile([128, 1], F32, tag="r_t")
            nc.vector.reciprocal(r_t[:], sum_t[:])
            r9_t = work_pool.tile([128, 1], F32, tag="r9_t")
            nc.vector.tensor_scalar_mul(r9_t[:], r_t[:], 1.0 - GAMMA)
            # p = e * (0.9/sum) + GAMMA/N_CAND
            pp_t = work_pool.tile([128, N_CAND], F32, tag="pp_t")
            nc.vector.tensor_scalar(
                pp_t[:], e_t[:], r9_t[:], MIX, op0=OP.mult, op1=OP.add
            )
            lp_t = work_pool.tile([128, N_CAND], F32, tag="lp_t")
            nc.scalar.activation(lp_t[:], pp_t[:], AF.Ln)

            # gumbel: g = -log(-log(u + 1e-20) + 1e-20) = -l2
            l1_t = work_pool.tile([128, N_CAND], F32, tag="l1_t")
            nc.scalar.activation(l1_t[:], u_t[:], AF.Ln, bias=eps_c[:], scale=1.0)
            l2_t = work_pool.tile([128, N_CAND], F32, tag="l2_t")
            nc.scalar.activation(l2_t[:], l1_t[:], AF.Ln, bias=eps_c[:], scale=-1.0)

            # keys = log(p) + g = lp - l2
            keys_t = work_pool.tile([128, N_CAND], F32, tag="keys_t")
            nc.vector.tensor_sub(keys_t[:], lp_t[:], l2_t[:])

            # ---- top-32 via 4 rounds of (max8, index8, replace) ---------------
            idx_t = work_pool.tile([128, K_OUT], U32, tag="idx_t")
            m8_t = work_pool.tile([128, 8], F32, tag="m8_t")
            for r in range(K_OUT // 8):
                nc.vector.max(out=m8_t[:], in_=keys_t[:])
                nc.vector.max_index(
                    out=idx_t[:, r * 8 : (r + 1) * 8],
                    in_max=m8_t[:],
                    in_values=keys_t[:],
                )
                if r < K_OUT // 8 - 1:
                    nc.vector.match_replace(
                        out=keys_t[:],
                        in_to_replace=m8_t[:],
                        in_values=keys_t[:],
                        imm_value=NEG_BIG,
                    )

            nc.scalar.dma_start(out_ap[r0 : r0 + 128, :], idx_t[:])

    nc.compile()
    return nc


_CACHE = {}


def _get_nc():
    if "nc" not in _CACHE:
        _CACHE["nc"] = build_nc()
    return _CACHE["nc"]


def kernel(
    target_embed, candidate_embeds, Wq, bq, Wk, bk, u
):  # full inputs -> full output
    nc = _get_nc()
    target_embed = np.ascontiguousarray(np.asarray(target_embed, dtype=np.float32))
    candidate_embeds = np.ascontiguousarray(
        np.asarray(candidate_embeds, dtype=np.float32)
    )
    Wq = np.ascontiguousarray(np.asarray(Wq, dtype=np.float32))
    bq = np.ascontiguousarray(np.asarray(bq, dtype=np.float32))
    Wk = np.ascontiguousarray(np.asarray(Wk, dtype=np.float32))
    bk = np.ascontiguousarray(np.asarray(bk, dtype=np.float32))
    u = np.ascontiguousarray(np.asarray(u, dtype=np.float32))

    in_maps = []
    for c in range(N_CORES):
        lo, hi = c * B_SHARD, (c + 1) * B_SHARD
        in_maps.append(
            {
                "target_embed": target_embed[lo:hi],
                "candidate_embeds": candidate_embeds[lo:hi],
                "Wq": Wq,
                "bq": bq,
                "Wk": Wk,
                "bk": bk,
                "u": u[lo:hi],
            }
        )

    res = run_bass_kernel_spmd(nc, in_maps, core_ids=list(range(N_CORES)))
    outs = [r["out"].astype(np.int32) for r in res.results]
    return np.concatenate(outs, axis=0)
